# revision 8
# baseline (speedup 1.0000x reference)
"""Single-head causal attention on 8 NeuronCores (batch-parallel), bf16.

x [8, 2048, 1024], Wq/Wk/Wv [1024, 64] -> out [8, 2048, 64].
One batch element per core. The host pre-transposes x to x.T (chunk-major
layout) and casts everything to bf16 (zero-flop marshalling), so the
device does no transposes at all:

  qkT[:,t]   = [Wq|Wk].T @ xT[:,t]      (qT rows 0:64, kT rows 64:128)
  v[t,:]     = xT[:,t-tile].T @ Wv      (natural [t,h] layout, PE direct)
  weiT[s,t]  = k[s]. q[t]              (lhsT = kT tile, rhs = qT cols)
  pT         = exp(weiT / sqrt(H))      (ACT, f32 psum -> bf16 sbuf,
                                         two s-tiles per instruction)
  out[t,h]   = sum_s pT[s,t] vaug[s,h]  (natural PV; ones column gives
                                         softmax denominators)
  out[t,h]  /= out[t,64]               (DVE reciprocal + scalar mul)

Causality via tile skipping, column-restricted diagonal score matmuls,
and one [128,128] triangular bf16 mask on diagonal blocks.

DMA plan: the three DMA queues (scalar/sync/gpsimd) each carry a slice
of every x chunk, and chunk n+1's dma_starts are gated on chunk n's
completion by a single gpsimd copy that reads one element from each
queue's chunk-n region and writes one element into each queue's
chunk-n+1 region (WAW dep -> semaphore wait at DMA issue).  This keeps
all DMA bandwidth focused on the earliest not-yet-landed chunk, so
chunk 0 lands ~4us earlier than a round-robin enqueue and the exp
stream starts correspondingly earlier.  The k-rebase projection for
chunk 0 is emitted in two 256-col halves so the first score pair can
issue between them.  Output DMAs ride the mostly-idle sync queue; the
last chunk's epilogue is split in half and its output DMA spread over
three queues to shorten the tail.
"""

from contextlib import ExitStack

import ml_dtypes
import numpy as np

import concourse.bass as bass
import concourse.mybir as mybir
import concourse.tile as tile
from concourse import bacc
from concourse.bass_utils import run_bass_kernel_spmd
from concourse.masks import make_upper_triangular

B, T, C, H = 8, 2048, 1024, 64
P = 128                      # partition tile
NT = T // P                  # 16 row tiles
NC = C // P                  # 8 contraction tiles
CH = 512                     # t-chunk width (psum bank)
NCH = T // CH                # 4 chunks
TPC = CH // P                # 4 t-tiles per chunk
VW = 66                      # vaug row stride: [v(64) | 1 | pad]

BF = mybir.dt.bfloat16
F32 = mybir.dt.float32
BF_NP = ml_dtypes.bfloat16

Exp = mybir.ActivationFunctionType.Exp
Copy = mybir.ActivationFunctionType.Copy

# per-queue c-tile split of each x chunk: {engine: (c_lo, c_hi)}.
# chunks 2/3 avoid the scalar queue: their gated dma_starts would sit in
# scalar program order ahead of the exp stream and stall it on the gate
# semaphore.  scalar's gates (chunks 0/1) all pass before the first exp.
XSPLIT = {
    0: {"scalar": (0, 2), "sync": (2, 5), "gpsimd": (5, 8)},
    1: {"scalar": (0, 3), "sync": (3, 6), "gpsimd": (6, 8)},
    2: {"sync": (0, 4), "gpsimd": (4, 8)},
    3: {"gpsimd": (0, 8)},
}


def build_kernel():
    nc = bacc.Bacc(
        "TRN2",
        target_bir_lowering=False,
        debug=False,
        enable_asserts=False,
        num_devices=B,
    )
    xTd = nc.dram_tensor("xT", [NCH, P, NC, CH], BF, kind="ExternalInput").ap()
    wqkd = nc.dram_tensor("Wqk", [P, NC, P], BF, kind="ExternalInput").ap()
    wvd = nc.dram_tensor("Wv", [P, NC, H], BF, kind="ExternalInput").ap()
    outd = nc.dram_tensor("out", [T, H], F32, kind="ExternalOutput").ap()

    with tile.TileContext(nc) as tc, ExitStack() as ctx:
        const = ctx.enter_context(tc.tile_pool(name="const", bufs=1))
        persist = ctx.enter_context(tc.tile_pool(name="persist", bufs=1))
        pt_p = ctx.enter_context(tc.tile_pool(name="pt", bufs=20))
        ost_p = ctx.enter_context(tc.tile_pool(name="ost", bufs=4))
        rc_p = ctx.enter_context(tc.tile_pool(name="rc", bufs=4))
        proj_ps = ctx.enter_context(tc.tile_pool(name="projps", bufs=2, space="PSUM"))
        wei_ps = ctx.enter_context(tc.tile_pool(name="weips", bufs=2, space="PSUM"))
        o_ps_p = ctx.enter_context(tc.tile_pool(name="ops", bufs=2, space="PSUM"))

        # PE p-state warmup: matmuls on a zeroed scratch tile (no DMA
        # dependency) keep the tensor engine running during the x DMA wait so
        # the clock has ramped to 2.4 GHz before the first real projection.
        garbage = const.tile([P, CH], BF, tag="garbage")
        nc.vector.memset(garbage, 0.0)
        warm_ps = proj_ps.tile([P, CH], F32, tag="ps")
        for _ in range(12):
            nc.tensor.matmul(warm_ps, garbage[:, 0:P], garbage, start=True, stop=True)

        xT = persist.tile([P, NC, T], BF, tag="xT")  # x.T: [c, t]
        wqk = const.tile([P, NC, P], BF, tag="wqk")
        wv = const.tile([P, NC, H], BF, tag="wv")

        # x load: per-queue FIFO chains.  Chunk 0 (plus the weights) is the
        # only work enqueued at t0, so it gets the full aggregate DMA
        # bandwidth; each later chunk is gated on the previous chunk's
        # completion by a gpsimd copy (below), not enqueued concurrently —
        # the DMA engines round-robin across all *enqueued* transfers, so an
        # ungated enqueue would steal bandwidth from the earliest chunk.
        def x_dma(ch):
            c0 = ch * CH
            eng = {"scalar": nc.scalar, "sync": nc.sync, "gpsimd": nc.gpsimd}
            for name, (lo, hi) in XSPLIT[ch].items():
                eng[name].dma_start(xT[:, lo:hi, c0 : c0 + CH], xTd[ch][:, lo:hi, :])

        nc.scalar.dma_start(wqk, wqkd)
        nc.sync.dma_start(wv, wvd)
        x_dma(0)

        # causal mask: gpsimd writes f32; DVE copy converts to bf16.
        # Emitted between chunk 0's dma issue and the gate copies so gpsimd's
        # in-order execution doesn't delay it behind the gate waits.
        scr_t = const.tile([P, P], F32, tag="scr_t")
        make_upper_triangular(nc, scr_t, val=1.0, diag=True)
        tri = const.tile([P, P], BF, tag="tri")  # tri[p,j]=1 iff j>=p
        nc.vector.tensor_copy(tri, scr_t)

        qkT = persist.tile([P, T], BF, tag="qkT")    # qT rows 0:64, kT 64:128
        kTlo = persist.tile([H, T], BF, tag="kTlo")  # kT re-based at partition 0
        vaug = persist.tile([P, NT, VW], BF, tag="vaug")  # [v | 1] per s-tile
        ones = nc.const_aps.scalar_like(1.0, vaug)
        nc.vector.tensor_copy(vaug[:, :, H : H + 1], ones.broadcast_to((P, NT, 1)))

        for ch in (1, 2, 3):
            g0 = (ch - 1) * CH
            c0 = ch * CH
            prev = XSPLIT[ch - 1]
            cur = XSPLIT[ch]
            # read last element of each queue's chunk ch-1 region (waits on
            # all the previous chunk's DMA completion semaphores), write the
            # first element of each queue's chunk ch region (WAW dep gates
            # each of chunk ch's dma_starts at issue).
            rd = [p[1] - 1 for p in prev.values()]
            wr = [p[0] for p in cur.values()]
            for i in range(max(len(rd), len(wr))):
                r = rd[min(i, len(rd) - 1)]
                w = wr[min(i, len(wr) - 1)]
                nc.gpsimd.tensor_copy(
                    xT[0:1, w, c0 : c0 + 1], xT[0:1, r, g0 + CH - 1 : g0 + CH]
                )
            x_dma(ch)

        def proj_qk(ch):
            chs = slice(ch * CH, (ch + 1) * CH)
            qk_ps = proj_ps.tile([P, CH], F32, tag="ps")
            for c in range(NC):
                nc.tensor.matmul(
                    qk_ps, wqk[:, c, :], xT[:, c, chs], start=(c == 0), stop=(c == NC - 1)
                )
            nc.vector.tensor_copy(qkT[0:P if ch else H, chs], qk_ps[0:P if ch else H, :])
            if ch == 0:
                # chunk 0's attention starts right after this projection; two
                # half-width PE k-projections into partitions 0:64 avoid
                # waiting on the SBUF->SBUF rebasing DMA's ~2.5us latency,
                # and the first score pair issues between the halves.
                return
            half = CH // 2
            lo = ch * CH
            if ch == 1:
                # rebase in two halves so s4/s5 land before the diagonal
                # score pair needs them
                nc.sync.dma_start(kTlo[:, lo : lo + half], qkT[H:P, lo : lo + half])
                nc.sync.dma_start(kTlo[:, lo + half : lo + CH], qkT[H:P, lo + half : lo + CH])
            else:
                nc.sync.dma_start(kTlo[:, chs], qkT[H:P, chs])

        def k0_half(h):
            half = CH // 2
            lo = h * half
            k_ps = proj_ps.tile([H, half], F32, tag="ps", name=f"k0_ps{h}")
            for c in range(NC):
                nc.tensor.matmul(
                    k_ps, wqk[:, c, H:P], xT[:, c, lo : lo + half],
                    start=(c == 0), stop=(c == NC - 1),
                )
            nc.vector.tensor_copy(kTlo[:, lo : lo + half], k_ps)

        def proj_v(ch):
            v_ps = proj_ps.tile([P, TPC, H], F32, tag="ps")
            for j in range(TPC):
                s = TPC * ch + j
                for c in range(NC):
                    nc.tensor.matmul(
                        v_ps[:, j, :],
                        xT[:, c, s * P : (s + 1) * P],
                        wv[:, c, :],
                        start=(c == 0),
                        stop=(c == NC - 1),
                    )
            nc.vector.tensor_copy(vaug[:, TPC * ch : TPC * ch + TPC, 0:H], v_ps)

        def emit_scores(ch, i, defer_muls=False):
            """Score matmuls + merged exp for s-tile pair (2i, 2i+1) of chunk ch."""
            base = ch * CH
            wei = wei_ps.tile([P, 2, CH], F32, tag="w")
            cols = []
            for u in range(2):
                s = 2 * i + u
                diag = s >= TPC * ch
                col0 = (s - TPC * ch) * P if diag else 0
                cols.append(col0)
                nc.tensor.matmul(
                    wei[:, u, col0:],
                    kTlo[:, s * P : (s + 1) * P],
                    qkT[0:H, base + col0 : base + CH],
                    start=True,
                    stop=True,
                )
            cmin = min(cols)
            pT = pt_p.tile([P, 2, CH], BF)
            # one ACT instruction covers both s-tiles; cols [cmin:col0) of a
            # diagonal tile hold exp(stale psum) — finite and never read.
            nc.scalar.activation(
                pT[:, :, cmin:], wei[:, :, cmin:], Exp, scale=float(H) ** -0.5
            )

            def muls():
                for u in range(2):
                    s = 2 * i + u
                    if s >= TPC * ch:
                        c0 = cols[u]
                        nc.vector.tensor_mul(
                            pT[:, u, c0 : c0 + P], pT[:, u, c0 : c0 + P], tri
                        )

            if not defer_muls:
                muls()
                muls = None
            return [(2 * i, cols[0], pT, 0), (2 * i + 1, cols[1], pT, 1)], muls

        def emit_pv(ch, o_ps, s, col0, pT, u):
            # start=True clears has_written for the WHOLE psum bank, so only
            # the first matmul of the chunk may set it; later slices' first
            # writes land on cleared bits and overwrite, then accumulate.
            for j in range(col0 // P, TPC):
                tj = TPC * ch + j
                nc.tensor.matmul(
                    o_ps[:, j, :],
                    pT[:, u, j * P : (j + 1) * P],
                    vaug[:, s, 0 : H + 1],
                    start=(s == 0 and j == 0),
                    stop=(s == tj),
                    skip_group_check=True,
                )

        def epilogue(ch, o_ps, jlo=0, jhi=TPC, engines=None):
            n = jhi - jlo
            rc = rc_p.tile([P, n, 1], F32)
            nc.vector.reciprocal(rc, o_ps[:, jlo:jhi, H : H + 1])
            ost = ost_p.tile([P, n, H], F32)
            nc.vector.tensor_mul(
                ost, o_ps[:, jlo:jhi, 0:H], rc.broadcast_to((P, n, H))
            )
            t0 = ch * CH + jlo * P
            if engines is None:
                nc.sync.dma_start(
                    outd[t0 : t0 + n * P, :].rearrange("(n p) h -> p n h", p=P), ost
                )
            else:
                # split across queues for the latency-critical tail
                for jj, eng in zip(range(n), engines):
                    tj = t0 + jj * P
                    eng.dma_start(
                        outd[tj : tj + P, :].rearrange("(n p) h -> p n h", p=P),
                        ost[:, jj : jj + 1, :],
                    )

        # Emission: scores/exp stream ahead, PV of chunk ch interleaves with
        # proj of chunk ch+1 so neither PE nor ACT starves.
        o_ps = {}
        pend = []

        def drain(n):
            while len(pend) > n:
                ch_, s_, c0_, pT_, u_ = pend.pop(0)
                emit_pv(ch_, o_ps[ch_], s_, c0_, pT_, u_)
                if ch_ == NCH - 1 and s_ == TPC * ch_ + TPC - 2:
                    # second-to-last s-tile: t-tiles jlo..jlo+1 are complete
                    epilogue(ch_, o_ps[ch_], 0, 2)
                elif s_ == TPC * ch_ + TPC - 1:
                    if ch_ == NCH - 1:
                        epilogue(
                            ch_, o_ps.pop(ch_), 2, TPC,
                            engines=[nc.gpsimd, nc.scalar],
                        )
                    else:
                        epilogue(ch_, o_ps.pop(ch_))

        proj_qk(0)
        o_ps[0] = o_ps_p.tile([P, TPC, H + 1], F32, tag="o", name="o_ps0")
        k0_half(0)
        # first score pair issues on the PE between the two k-projection
        # halves; its mask-muls (which wait on the first exp) are emitted
        # after the k0b cast so they don't block it in vector program order.
        p00, muls00 = emit_scores(0, 0, defer_muls=True)
        k0_half(1)
        p01, _ = emit_scores(0, 1)
        muls00()
        pend.extend((0, *e) for e in p00)
        pend.extend((0, *e) for e in p01)
        proj_qk(1)
        for ch in range(1, NCH):
            o_ps[ch] = o_ps_p.tile([P, TPC, H + 1], F32, tag="o", name=f"o_ps{ch}")
            npairs = (TPC * ch + TPC) // 2
            for i in range(npairs):
                for e, _ in [emit_scores(ch, i)]:
                    pend.extend((ch, *x) for x in e)
                if ch == 1 and i == 0:
                    proj_v(0)
                if i == 2 * ch and ch + 1 < NCH:
                    proj_qk(ch + 1)
                drain(3)
                if i == 2:
                    proj_v(ch)
        drain(0)

    nc.compile()
    return nc


_NC = None


def kernel(x, Wq, Wk, Wv, **run_kwargs):
    global _NC
    if _NC is None:
        _NC = build_kernel()
    x = np.asarray(x, dtype=np.float32)
    wqk = np.concatenate(
        [np.asarray(Wq, np.float32), np.asarray(Wk, np.float32)], axis=1
    ).astype(BF_NP)
    wqk_t = np.ascontiguousarray(wqk.reshape(NC, P, P).transpose(1, 0, 2))
    wv_t = np.ascontiguousarray(
        np.asarray(Wv, np.float32).astype(BF_NP).reshape(NC, P, H).transpose(1, 0, 2)
    )
    in_maps = []
    for b in range(B):
        xT = x[b].T.astype(BF_NP)  # [C, T]
        # chunk-major tiled layout: [NCH, P, NC, CH], 8 KiB contiguous lines
        xT_t = np.ascontiguousarray(
            xT.reshape(NC, P, NCH, CH).transpose(2, 1, 0, 3)
        )
        in_maps.append({"xT": xT_t, "Wqk": wqk_t, "Wv": wv_t})
    res = run_bass_kernel_spmd(_NC, in_maps, core_ids=list(range(B)), **run_kwargs)
    out = np.stack([res.results[b]["out"] for b in range(B)])
    if run_kwargs:
        kernel.last_result = res
    return out


if __name__ == "__main__":
    rng = np.random.default_rng(0)
    ins = {
        "x": rng.standard_normal((B, T, C), dtype=np.float32),
        "Wq": rng.standard_normal((C, H), dtype=np.float32) / np.sqrt(C),
        "Wk": rng.standard_normal((C, H), dtype=np.float32) / np.sqrt(C),
        "Wv": rng.standard_normal((C, H), dtype=np.float32) / np.sqrt(C),
    }
    out = kernel(**ins)
    print("out", out.shape, out.dtype)


# revision 13
# speedup vs baseline: 1.0200x; 1.0200x over previous
"""Single-head causal attention on 8 NeuronCores (batch-parallel), bf16.

x [8, 2048, 1024], Wq/Wk/Wv [1024, 64] -> out [8, 2048, 64].
One batch element per core. The host pre-transposes x to x.T (chunk-major
layout) and casts everything to bf16 (zero-flop marshalling), so the
device does no transposes at all:

  qkT[:,t]   = [Wq|Wk].T @ xT[:,t]      (qT rows 0:64, kT rows 64:128)
  v[t,:]     = xT[:,t-tile].T @ Wv      (natural [t,h] layout, PE direct)
  weiT[s,t]  = k[s]. q[t]              (lhsT = kT tile, rhs = qT cols)
  pT         = exp(weiT / sqrt(H))      (ACT, f32 psum -> bf16 sbuf,
                                         two s-tiles per instruction)
  out[t,h]   = sum_s pT[s,t] vaug[s,h]  (natural PV; ones column gives
                                         softmax denominators)
  out[t,h]  /= out[t,64]               (DVE reciprocal + scalar mul)

Causality via tile skipping, column-restricted diagonal score matmuls,
and one [128,128] triangular bf16 mask on diagonal blocks.

DMA plan: the three DMA queues (scalar/sync/gpsimd) each carry a slice
of every x chunk, and chunk n+1's dma_starts are gated on chunk n's
completion by a single gpsimd copy that reads one element from each
queue's chunk-n region and writes one element into each queue's
chunk-n+1 region (WAW dep -> semaphore wait at DMA issue).  This keeps
all DMA bandwidth focused on the earliest not-yet-landed chunk, so
chunk 0 lands ~4us earlier than a round-robin enqueue and the exp
stream starts correspondingly earlier.  The k-rebase projection for
chunk 0 is emitted in two 256-col halves so the first score pair can
issue between them.  Output DMAs ride the mostly-idle sync queue; the
last chunk's epilogue is split in half and its output DMA spread over
three queues to shorten the tail.
"""

from contextlib import ExitStack

import ml_dtypes
import numpy as np

import concourse.bass as bass
import concourse.mybir as mybir
import concourse.tile as tile
from concourse import bacc
from concourse.bass_utils import run_bass_kernel_spmd
from concourse.masks import make_upper_triangular

B, T, C, H = 8, 2048, 1024, 64
P = 128                      # partition tile
NT = T // P                  # 16 row tiles
NC = C // P                  # 8 contraction tiles
CH = 512                     # t-chunk width (psum bank)
NCH = T // CH                # 4 chunks
TPC = CH // P                # 4 t-tiles per chunk
VW = 66                      # vaug row stride: [v(64) | 1 | pad]

BF = mybir.dt.bfloat16
F32 = mybir.dt.float32
BF_NP = ml_dtypes.bfloat16

Exp = mybir.ActivationFunctionType.Exp
Copy = mybir.ActivationFunctionType.Copy

# per-queue c-tile split of each x chunk: {engine: (c_lo, c_hi)}.
# Each queue carries a balanced slice of every chunk, chained in chunk
# order (a queue's chunk-n+1 slice is gated on its own chunk-n slice), so
# all three queues stream continuously at full aggregate bandwidth while
# chunks complete in order.  The scalar queue's chunk 2/3 dma_starts are
# emitted inline in the exp stream at points where their gate semaphores
# have already passed, so they never stall an exp.
XSPLIT = {
    0: {"scalar": (0, 2), "sync": (2, 5), "gpsimd": (5, 8)},
    1: {"scalar": (0, 3), "sync": (3, 6), "gpsimd": (6, 8)},
    2: {"scalar": (0, 3), "sync": (3, 5), "gpsimd": (5, 8)},
    3: {"scalar": (0, 2), "sync": (2, 5), "gpsimd": (5, 8)},
}


def build_kernel():
    nc = bacc.Bacc(
        "TRN2",
        target_bir_lowering=False,
        debug=False,
        enable_asserts=False,
        num_devices=B,
    )
    xTd = nc.dram_tensor("xT", [NCH, P, NC, CH], BF, kind="ExternalInput").ap()
    wqkd = nc.dram_tensor("Wqk", [P, NC, P], BF, kind="ExternalInput").ap()
    wvd = nc.dram_tensor("Wv", [P, NC, H], BF, kind="ExternalInput").ap()
    outd = nc.dram_tensor("out", [T, H], F32, kind="ExternalOutput").ap()

    with tile.TileContext(nc) as tc, ExitStack() as ctx:
        const = ctx.enter_context(tc.tile_pool(name="const", bufs=1))
        persist = ctx.enter_context(tc.tile_pool(name="persist", bufs=1))
        pt_p = ctx.enter_context(tc.tile_pool(name="pt", bufs=20))
        ost_p = ctx.enter_context(tc.tile_pool(name="ost", bufs=4))
        rc_p = ctx.enter_context(tc.tile_pool(name="rc", bufs=4))
        proj_ps = ctx.enter_context(tc.tile_pool(name="projps", bufs=2, space="PSUM"))
        wei_ps = ctx.enter_context(tc.tile_pool(name="weips", bufs=2, space="PSUM"))
        o_ps_p = ctx.enter_context(tc.tile_pool(name="ops", bufs=2, space="PSUM"))

        # PE p-state warmup: matmuls on a zeroed scratch tile (no DMA
        # dependency) keep the tensor engine running during the x DMA wait so
        # the clock has ramped to 2.4 GHz (and stays there) until the first
        # real projection is ready.
        garbage = const.tile([P, CH], BF, tag="garbage")
        nc.vector.memset(garbage, 0.0)
        warm_ps = proj_ps.tile([P, CH], F32, tag="ps")
        for _ in range(17):
            nc.tensor.matmul(warm_ps, garbage[:, 0:P], garbage, start=True, stop=True)

        xT = persist.tile([P, NC, T], BF, tag="xT")  # x.T: [c, t]
        wqk = const.tile([P, NC, P], BF, tag="wqk")
        wv = const.tile([P, NC, H], BF, tag="wv")
        gate_scr = const.tile([1, 4], BF, tag="gate_scr")  # gate-dma dst

        def x_slice(eng_name, ch):
            lo, hi = XSPLIT[ch][eng_name]
            c0 = ch * CH
            return xT[:, lo:hi, c0 : c0 + CH], xTd[ch][:, lo:hi, :]

        def x_part_tail(eng_name, ch):
            """last element of a queue's chunk-ch slice (sem-carrying read)"""
            lo, hi = XSPLIT[ch][eng_name]
            return xT[0:1, hi - 1, ch * CH + CH - 1 : ch * CH + CH]

        def scalar_x(ch):
            # activation-copy gate on scalar's own previous slice, then dma.
            # Emitted at a point in scalar program order where the gate
            # semaphore has already (or harmlessly) passed.
            nc.scalar.activation(gate_scr[0:1, 0:1], x_part_tail("scalar", ch - 1), Copy)
            nc.scalar.dma_start(*x_slice("scalar", ch))

        def sync_x(ch):
            # sync cannot run compute ops; a 1-element sbuf->sbuf dma carries
            # the read-dependency semaphore wait at issue, and the engine's
            # in-order issue then gates the real dma behind it.
            nc.sync.dma_start(gate_scr[0:1, 1:2], x_part_tail("sync", ch - 1))
            nc.sync.dma_start(*x_slice("sync", ch))

        def gpsimd_x(ch):
            nc.gpsimd.tensor_copy(gate_scr[0:1, 2:3], x_part_tail("gpsimd", ch - 1))
            nc.gpsimd.dma_start(*x_slice("gpsimd", ch))

        nc.scalar.dma_start(wqk, wqkd)
        nc.sync.dma_start(wv, wvd)
        nc.scalar.dma_start(*x_slice("scalar", 0))
        nc.sync.dma_start(*x_slice("sync", 0))
        nc.gpsimd.dma_start(*x_slice("gpsimd", 0))

        # causal mask: gpsimd writes f32; DVE copy converts to bf16.
        # Emitted between chunk 0's dma issue and the gate copies so gpsimd's
        # in-order execution doesn't delay it behind the gate waits.
        scr_t = const.tile([P, P], F32, tag="scr_t")
        make_upper_triangular(nc, scr_t, val=1.0, diag=True)
        tri = const.tile([P, P], BF, tag="tri")  # tri[p,j]=1 iff j>=p
        nc.vector.tensor_copy(tri, scr_t)

        qkT = persist.tile([P, T], BF, tag="qkT")    # qT rows 0:64, kT 64:128
        kTlo = persist.tile([H, T], BF, tag="kTlo")  # kT re-based at partition 0
        vaug = persist.tile([P, NT, VW], BF, tag="vaug")  # [v | 1] per s-tile
        ones = nc.const_aps.scalar_like(1.0, vaug)
        nc.vector.tensor_copy(vaug[:, :, H : H + 1], ones.broadcast_to((P, NT, 1)))

        # sync + gpsimd x chains (all gates pass while those engines are
        # otherwise idle); scalar's chunk 1 gate passes before the first exp
        # so it is safe to emit here too.  scalar's chunk 2/3 dmas and sync's
        # chunk 3 dma are emitted later, inline in the compute stream, so
        # their gate waits never block the exp stream or the k-rebases.
        scalar_x(1)
        sync_x(1)
        gpsimd_x(1)
        sync_x(2)
        gpsimd_x(2)
        gpsimd_x(3)

        def proj_qk(ch):
            chs = slice(ch * CH, (ch + 1) * CH)
            qk_ps = proj_ps.tile([P, CH], F32, tag="ps")
            for c in range(NC):
                nc.tensor.matmul(
                    qk_ps, wqk[:, c, :], xT[:, c, chs], start=(c == 0), stop=(c == NC - 1)
                )
            nc.vector.tensor_copy(qkT[0:P if ch else H, chs], qk_ps[0:P if ch else H, :])
            if ch == 0:
                # chunk 0's attention starts right after this projection; two
                # half-width PE k-projections into partitions 0:64 avoid
                # waiting on the SBUF->SBUF rebasing DMA's ~2.5us latency,
                # and the first score pair issues between the halves.
                return
            half = CH // 2
            lo = ch * CH
            if ch == 1:
                # rebase in two halves so s4/s5 land before the diagonal
                # score pair needs them
                nc.sync.dma_start(kTlo[:, lo : lo + half], qkT[H:P, lo : lo + half])
                nc.sync.dma_start(kTlo[:, lo + half : lo + CH], qkT[H:P, lo + half : lo + CH])
            else:
                nc.sync.dma_start(kTlo[:, chs], qkT[H:P, chs])

        def k0_half(h):
            half = CH // 2
            lo = h * half
            k_ps = proj_ps.tile([H, half], F32, tag="ps", name=f"k0_ps{h}")
            for c in range(NC):
                nc.tensor.matmul(
                    k_ps, wqk[:, c, H:P], xT[:, c, lo : lo + half],
                    start=(c == 0), stop=(c == NC - 1),
                )
            nc.vector.tensor_copy(kTlo[:, lo : lo + half], k_ps)

        def proj_v(ch):
            v_ps = proj_ps.tile([P, TPC, H], F32, tag="ps")
            for j in range(TPC):
                s = TPC * ch + j
                for c in range(NC):
                    nc.tensor.matmul(
                        v_ps[:, j, :],
                        xT[:, c, s * P : (s + 1) * P],
                        wv[:, c, :],
                        start=(c == 0),
                        stop=(c == NC - 1),
                    )
            nc.vector.tensor_copy(vaug[:, TPC * ch : TPC * ch + TPC, 0:H], v_ps)

        def emit_scores(ch, i, defer_muls=False):
            """Score matmuls + merged exp for s-tile pair (2i, 2i+1) of chunk ch."""
            base = ch * CH
            wei = wei_ps.tile([P, 2, CH], F32, tag="w")
            cols = []
            for u in range(2):
                s = 2 * i + u
                diag = s >= TPC * ch
                col0 = (s - TPC * ch) * P if diag else 0
                cols.append(col0)
                nc.tensor.matmul(
                    wei[:, u, col0:],
                    kTlo[:, s * P : (s + 1) * P],
                    qkT[0:H, base + col0 : base + CH],
                    start=True,
                    stop=True,
                )
            cmin = min(cols)
            pT = pt_p.tile([P, 2, CH], BF)
            # one ACT instruction covers both s-tiles; cols [cmin:col0) of a
            # diagonal tile hold exp(stale psum) — finite and never read.
            nc.scalar.activation(
                pT[:, :, cmin:], wei[:, :, cmin:], Exp, scale=float(H) ** -0.5
            )

            def muls():
                for u in range(2):
                    s = 2 * i + u
                    if s >= TPC * ch:
                        c0 = cols[u]
                        nc.vector.tensor_mul(
                            pT[:, u, c0 : c0 + P], pT[:, u, c0 : c0 + P], tri
                        )

            if not defer_muls:
                muls()
                muls = None
            return [(2 * i, cols[0], pT, 0), (2 * i + 1, cols[1], pT, 1)], muls

        def emit_pv(ch, o_ps, s, col0, pT, u):
            # start=True clears has_written for the WHOLE psum bank, so only
            # the first matmul of the chunk may set it; later slices' first
            # writes land on cleared bits and overwrite, then accumulate.
            for j in range(col0 // P, TPC):
                tj = TPC * ch + j
                nc.tensor.matmul(
                    o_ps[:, j, :],
                    pT[:, u, j * P : (j + 1) * P],
                    vaug[:, s, 0 : H + 1],
                    start=(s == 0 and j == 0),
                    stop=(s == tj),
                    skip_group_check=True,
                )

        def epilogue(ch, o_ps, jlo=0, jhi=TPC, engines=None):
            n = jhi - jlo
            rc = rc_p.tile([P, n, 1], F32)
            nc.vector.reciprocal(rc, o_ps[:, jlo:jhi, H : H + 1])
            ost = ost_p.tile([P, n, H], F32)
            nc.vector.tensor_mul(
                ost, o_ps[:, jlo:jhi, 0:H], rc.broadcast_to((P, n, H))
            )
            t0 = ch * CH + jlo * P
            if engines is None:
                nc.sync.dma_start(
                    outd[t0 : t0 + n * P, :].rearrange("(n p) h -> p n h", p=P), ost
                )
            else:
                # split across queues for the latency-critical tail
                for jj, eng in zip(range(n), engines):
                    tj = t0 + jj * P
                    eng.dma_start(
                        outd[tj : tj + P, :].rearrange("(n p) h -> p n h", p=P),
                        ost[:, jj : jj + 1, :],
                    )

        # Emission: scores/exp stream ahead, PV of chunk ch interleaves with
        # proj of chunk ch+1 so neither PE nor ACT starves.
        o_ps = {}
        pend = []

        def drain(n):
            while len(pend) > n:
                ch_, s_, c0_, pT_, u_ = pend.pop(0)
                emit_pv(ch_, o_ps[ch_], s_, c0_, pT_, u_)
                if ch_ == NCH - 1 and s_ == TPC * ch_ + TPC - 2:
                    # second-to-last s-tile: t-tiles jlo..jlo+1 are complete
                    epilogue(ch_, o_ps[ch_], 0, 2)
                elif s_ == TPC * ch_ + TPC - 1:
                    if ch_ == NCH - 1:
                        epilogue(
                            ch_, o_ps.pop(ch_), 2, TPC,
                            engines=[nc.gpsimd, nc.scalar],
                        )
                    else:
                        epilogue(ch_, o_ps.pop(ch_))

        proj_qk(0)
        o_ps[0] = o_ps_p.tile([P, TPC, H + 1], F32, tag="o", name="o_ps0")
        k0_half(0)
        # first score pair issues on the PE between the two k-projection
        # halves; its mask-muls (which wait on the first exp) are emitted
        # after the k0b cast so they don't block it in vector program order.
        p00, muls00 = emit_scores(0, 0, defer_muls=True)
        k0_half(1)
        p01, _ = emit_scores(0, 1)
        scalar_x(2)  # gate passed by now; fills the pre-chunk-1 exp bubble
        muls00()
        pend.extend((0, *e) for e in p00)
        pend.extend((0, *e) for e in p01)
        proj_qk(1)
        sync_x(3)  # after the chunk-1 rebases in sync program order
        for ch in range(1, NCH):
            o_ps[ch] = o_ps_p.tile([P, TPC, H + 1], F32, tag="o", name=f"o_ps{ch}")
            npairs = (TPC * ch + TPC) // 2
            for i in range(npairs):
                for e, _ in [emit_scores(ch, i)]:
                    pend.extend((ch, *x) for x in e)
                if ch == 1 and i == 0:
                    proj_v(0)
                if ch == 1 and i == 3:
                    scalar_x(3)
                if i == 2 * ch and ch + 1 < NCH:
                    proj_qk(ch + 1)
                drain(3)
                if i == 2:
                    proj_v(ch)
        drain(0)

    nc.compile()
    return nc


_NC = None


def kernel(x, Wq, Wk, Wv, **run_kwargs):
    global _NC
    if _NC is None:
        _NC = build_kernel()
    x = np.asarray(x, dtype=np.float32)
    wqk = np.concatenate(
        [np.asarray(Wq, np.float32), np.asarray(Wk, np.float32)], axis=1
    ).astype(BF_NP)
    wqk_t = np.ascontiguousarray(wqk.reshape(NC, P, P).transpose(1, 0, 2))
    wv_t = np.ascontiguousarray(
        np.asarray(Wv, np.float32).astype(BF_NP).reshape(NC, P, H).transpose(1, 0, 2)
    )
    in_maps = []
    for b in range(B):
        xT = x[b].T.astype(BF_NP)  # [C, T]
        # chunk-major tiled layout: [NCH, P, NC, CH], 8 KiB contiguous lines
        xT_t = np.ascontiguousarray(
            xT.reshape(NC, P, NCH, CH).transpose(2, 1, 0, 3)
        )
        in_maps.append({"xT": xT_t, "Wqk": wqk_t, "Wv": wv_t})
    res = run_bass_kernel_spmd(_NC, in_maps, core_ids=list(range(B)), **run_kwargs)
    out = np.stack([res.results[b]["out"] for b in range(B)])
    if run_kwargs:
        kernel.last_result = res
    return out


if __name__ == "__main__":
    rng = np.random.default_rng(0)
    ins = {
        "x": rng.standard_normal((B, T, C), dtype=np.float32),
        "Wq": rng.standard_normal((C, H), dtype=np.float32) / np.sqrt(C),
        "Wk": rng.standard_normal((C, H), dtype=np.float32) / np.sqrt(C),
        "Wv": rng.standard_normal((C, H), dtype=np.float32) / np.sqrt(C),
    }
    out = kernel(**ins)
    print("out", out.shape, out.dtype)
